# revision 22
# baseline (speedup 1.0000x reference)
"""Trainium2 Bass kernel for nn_CPCModel_50878182588587 (vq_codebook).

Computes, for inputs encodedData [B,N,D] and protos [K,D]:
  pass1: FCM memberships of v vs protos (p=2), x = 0.5*v + 0.5*(belong@protos)
  pass2: FCM memberships of x vs protos (p=2)  -> output [B,N,K]

Sharding: data-parallel over B across 8 NeuronCores; protos replicated.

v2 design (per core, T=8192 tokens, macro-tiles of 512 tokens):
  - Host supplies vth = 0.5*v^T in bf16 (layout prep) and the aug row
    [||v||^2/4 ; 1], so the kernel has no transposes / input squares.
  - sq = ||v||^2 + ||c||^2 - 2 v.c formed entirely in PSUM via
    augmented-contraction matmul rows; 1/sq via DVE reciprocal_approx_fast
    (sq bounded away from 0 here: sq1 in [170,351], sq2 in [42,91]).
  - Orientation B (K/D on partitions, tokens free) for dist1/target;
    orientation A (tokens on partitions) for dist2 so s2 reduction and
    the final normalize are per-partition ops and the out DMA is contiguous.
  - Final normalize on the Activation engine (Copy with per-partition
    scale) -- the Pool/GpSimd engine is pathologically slow and unused.
  - Issue order is software-pipelined three stages deep:
    F(im) = dist1+srow+tgt+bcq+bcs, M(im) = xt/x2row/aug2l,
    B(im) = dist2/recip/normalize/out, issued F0,F1,M0,F2,M1,B0,... so
    TensorE never waits on the cross-engine reduction chains.
  - Output in bf16 (harness gate is 2e-2; bf16 adds ~2e-3 max), host
    casts back to f32.
"""

import sys

import numpy as np

sys.path.insert(0, "/opt/trn_rl_repo")

import concourse.bass as bass  # noqa: E402
from concourse import bacc  # noqa: E402
import concourse.mybir as mybir  # noqa: E402
import concourse.tile as tile  # noqa: E402

B, N, D, K = 64, 1024, 256, 512
NCORES = 8
MACRO = 512  # tokens per macro-tile
f32 = mybir.dt.float32
bf16 = mybir.dt.bfloat16
fp8 = mybir.dt.float8e4
FT = mybir.ActivationFunctionType
DR = mybir.MatmulPerfMode.DoubleRow


def recip_fast(nc, out, in_):
    """reciprocal_approx_fast with any output dtype (wrapper asserts fp32)."""
    from concourse.dve_ops import RECIP_APPROX_FAST_CONSTS, RECIPROCAL_APPROX_FAST

    c = RECIP_APPROX_FAST_CONSTS
    return nc.vector._custom_dve(
        RECIPROCAL_APPROX_FAST, out=out, in0=in_, s0=c["s0"], s1=c["s1"], imm2=c["imm2"]
    )


# one-Newton approximate reciprocal with free-dim accumulate: out = ~1/in0,
# accum_out = sum(out).  Seed constants tuned for the [-4.5,-4] interval of
# x*bitcast(~x); max rel err ~2e-3 -- fine for the normalize-only use.
RECIP1NR_C0 = -4.0 / 17.0 * 1.004
RECIP1NR_C1 = 2.0 * 1.001


def _register_recip_acc():
    from operator import add as _add

    from concourse import dve_ops as D
    from concourse.dve_spec import C0, C1, AluOp, Bin, Spec, Src0, Zero, lower
    from concourse.dve_uop import DveOpSpec

    for op in D.OPS:
        if op.name == "RECIP1NR_ACC_ANT":
            return op

    def _ref(in0, in1, c0, c1, c2):
        import numpy as np

        nx = (~np.asarray(in0, np.float32).view(np.int32)).view(np.float32)
        y0 = nx * c0
        body = y0 * (c1 - in0 * y0)
        return body, body.reshape(body.shape[0], -1).sum(axis=-1)

    _not = Bin(AluOp.BITWISE_NOT, Src0, Src0)
    _y0 = _not * C0
    spec = Spec(
        body=_y0 * (C1 - Src0 * _y0),
        accum=_add,
        accum_init=Zero,
        reference=_ref,
    )
    row = max(D._SUB_OPCODE_FOR_NAME.values()) + 1
    name = "RECIP1NR_ACC_ANT"
    uops = lower(spec, ver="v3")
    sha = DveOpSpec(name=name, opcode=row, uops=uops, rd1_en=False).sha("v3")
    op = D.DveOp(name, spec, subdim=False, uops_sha={"v3": sha})
    D.OPS.append(op)
    D.CUSTOM_DVE_SPECS[name] = spec
    D._SUB_OPCODE_FOR_NAME[name] = row
    return op


def recip_acc(nc, out, in_, accum_out):
    op = _register_recip_acc()
    return nc.vector._custom_dve(
        op,
        out=out,
        in0=in_,
        s0=RECIP1NR_C0,
        s1=RECIP1NR_C1,
        imm2=0.0,
        accum_out=accum_out,
    )


def _patch_walrus_args():
    from concourse import bass_utils as BU

    if getattr(BU, "_ant_ldwopt_patched", False):
        return
    orig = BU.get_walrus_args

    def patched(*a, **kw):
        return [*orig(*a, **kw), "--enable-ldw-opt=false"]

    BU.get_walrus_args = patched
    BU._ant_ldwopt_patched = True


_patch_walrus_args()


def build_bass(T, do_compile=True, out_dtype=bf16):
    assert T % MACRO == 0
    nmacro = T // MACRO
    nc = bacc.Bacc(trn_type="TRN2")

    vth_d = nc.dram_tensor("vth", [2, 128, T], bf16, kind="ExternalInput")  # 0.5 v^T
    # fp8 DoubleRow operands: [p, i, *] pairs i=0/1 are contraction rows p,
    # p+128.  vthq = v^T/8, ptm4q = -p^T/4 -> cross matmul yields -2 v.c / 64.
    vthq_d = nc.dram_tensor("vthq", [128, 2, T], fp8, kind="ExternalInput")
    ptm4q_d = nc.dram_tensor("ptm4q", [128, 2, K], fp8, kind="ExternalInput")
    # pnq[g][p, i, d] = 4*protos[g*256 + i*128 + p, d] for target GEMM
    pnq_d = nc.dram_tensor("pnq", [2, 128, 2, D], fp8, kind="ExternalInput")
    # eights column for the s-row DoubleRow reduction (8 * w1' sums to 512*s1)
    consts8_d = nc.dram_tensor("consts8", [128, 2, 16], fp8, kind="ExternalInput")
    aug1r_d = nc.dram_tensor("aug1r", [2, T], bf16, kind="ExternalInput")  # v2q;1
    ptm2_d = nc.dram_tensor("ptm2", [D, K], bf16, kind="ExternalInput")  # -2*protos.T
    # aug1l rows: [0]=4.0/64 (scales v2q back to v2/64), [1]=c2/64
    aug1l_d = nc.dram_tensor("aug1l", [2, K], bf16, kind="ExternalInput")
    # aug2r rows: [0]=1.0 (x2 row), [1]=c2
    aug2r_d = nc.dram_tensor("aug2r", [2, K], bf16, kind="ExternalInput")
    rowinit_d = nc.dram_tensor("rowinit", [2, MACRO], bf16, kind="ExternalInput")
    consts_d = nc.dram_tensor("consts", [128, 2], bf16, kind="ExternalInput")  # 1s, 2s
    onesrow_d = nc.dram_tensor("onesrow", [1, 128], bf16, kind="ExternalInput")
    out_d = nc.dram_tensor("out", [T, K], out_dtype, kind="ExternalOutput")

    NCHUNK = 8  # input DMA split along T so macro 0 starts early
    with tile.TileContext(nc) as tc:
        with (
            tc.tile_pool(name="singles", bufs=1) as singles,
            tc.tile_pool(name="wt", bufs=6) as wtp,
            tc.tile_pool(name="bcs", bufs=3) as bcsp,
            tc.tile_pool(name="isn", bufs=2) as isnp,
            tc.tile_pool(name="th", bufs=2) as thp,
            tc.tile_pool(name="xt", bufs=6) as xtp,
            tc.tile_pool(name="sqv", bufs=2) as sqvp,
            tc.tile_pool(name="w2", bufs=4) as w2p,
            tc.tile_pool(name="ob", bufs=3) as obp,
            tc.tile_pool(name="small", bufs=16) as smallp,
            tc.tile_pool(name="sqp", bufs=2, space="PSUM") as sq_ps,
            tc.tile_pool(name="tgp", bufs=2, space="PSUM") as tg_ps,
            tc.tile_pool(name="ps2p", bufs=3, space="PSUM") as ps2_ps,
            tc.tile_pool(name="rwp", bufs=1, space="PSUM") as rows_ps,
        ):
            # ---- statics ----
            ptm2_sb = []
            for d2 in range(2):
                t2 = singles.tile([128, K], bf16, tag=f"ptm2_{d2}")
                nc.sync.dma_start(out=t2, in_=ptm2_d[d2 * 128 : (d2 + 1) * 128, :])
                ptm2_sb.append(t2)
            ptm4q_sb = singles.tile([128, 2, K], fp8, tag="ptm4q")
            nc.sync.dma_start(out=ptm4q_sb, in_=ptm4q_d[:, :, :])
            pnq_sb = []
            for g in range(2):
                t = singles.tile([128, 2, D], fp8, tag=f"pnq_{g}", name=f"pnq_{g}")
                nc.sync.dma_start(out=t, in_=pnq_d[g, :, :, :])
                pnq_sb.append(t)
            consts8_sb = singles.tile([128, 2, 16], fp8, tag="consts8")
            nc.sync.dma_start(out=consts8_sb, in_=consts8_d[:, :, :])
            aug1l_sb = singles.tile([2, K], bf16, tag="aug1l")
            nc.sync.dma_start(out=aug1l_sb, in_=aug1l_d[:, :])
            aug2r_sb = singles.tile([2, K], bf16, tag="aug2r")
            nc.sync.dma_start(out=aug2r_sb, in_=aug2r_d[:, :])
            consts_sb = singles.tile([128, 2], bf16, tag="consts")
            nc.sync.dma_start(out=consts_sb, in_=consts_d[:, :])
            onesrow_sb = singles.tile([1, 128], bf16, tag="onesrow")
            nc.sync.dma_start(out=onesrow_sb, in_=onesrow_d[:, :])
            # aug2l tiles (row0 = x2, rewritten per macro; row1 static ones)
            aug2l_sb = []
            for e in range(2):
                t = singles.tile([2, MACRO], bf16, tag=f"aug2l_{e}")
                nc.sync.dma_start(out=t, in_=rowinit_d[:, :])
                aug2l_sb.append(t)
            # full input resident in SBUF; chunked DMAs interleaved across
            # d2 so macro 0's dependencies land first
            aug1r_sb = singles.tile([2, T], bf16, tag="aug1r")
            nc.sync.dma_start(out=aug1r_sb, in_=aug1r_d[:, :])
            vth_sb = [
                singles.tile([128, T], bf16, tag=f"vth_{d2}", name=f"vth_{d2}")
                for d2 in range(2)
            ]
            vthq_sb = singles.tile([128, 2, T], fp8, tag="vthq")
            cs = T // NCHUNK
            for ch in range(NCHUNK):
                nc.sync.dma_start(
                    out=vthq_sb[:, :, ch * cs : (ch + 1) * cs],
                    in_=vthq_d[:, :, ch * cs : (ch + 1) * cs],
                )
                for d2 in range(2):
                    nc.sync.dma_start(
                        out=vth_sb[d2][:, ch * cs : (ch + 1) * cs],
                        in_=vth_d[d2, :, ch * cs : (ch + 1) * cs],
                    )
            ones_col = consts_sb[:, 0:1]

            st = [dict() for _ in range(nmacro)]  # per-macro state

            def front(im):
                s = st[im]
                tok0 = im * MACRO
                vsl = [vth_sb[d2][:, tok0 : tok0 + MACRO] for d2 in range(2)]
                # rows psum: [0:1]=2*s1 row, [32:33]=x2 row (written in back)
                rows = rows_ps.tile([33, MACRO], f32, tag="rows")
                s["rows"] = rows
                # dist1 (fp8 DoubleRow cross + bf16 aug) -> w1' = 64/sq1 in fp8
                # pairs; srow (8*w1' -> 512*s1) and target' (= 256*tgt) via
                # DoubleRow too.  The recip of srow then gives exactly the
                # 0.5/s1 scale the x update needs.
                vq = vthq_sb[:, :, tok0 : tok0 + MACRO]
                tg = [
                    tg_ps.tile([128, MACRO], f32, tag="tg", name=f"tg_{d2}")
                    for d2 in range(2)
                ]
                w1q = [None, None]
                for kc in range(4):
                    g, ip = kc // 2, kc % 2
                    sqp = sq_ps.tile([128, MACRO], f32, tag="sq")
                    nc.tensor.matmul(
                        sqp,
                        ptm4q_sb[:, :, kc * 128 : (kc + 1) * 128],
                        vq,
                        start=True,
                        stop=False,
                        perf_mode=DR,
                    )
                    nc.tensor.matmul(
                        sqp,
                        aug1l_sb[:, kc * 128 : (kc + 1) * 128],
                        aug1r_sb[:, tok0 : tok0 + MACRO],
                        start=False,
                        stop=True,
                    )
                    if ip == 0:
                        w1q[g] = wtp.tile(
                            [128, 2, MACRO], fp8, tag="wt", name=f"w1q_{g}"
                        )
                    recip_fast(nc, w1q[g][:, ip, :], sqp)
                    if ip == 1:
                        nc.tensor.matmul(
                            rows[0:1, :],
                            consts8_sb[:, :, 0:1],
                            w1q[g],
                            start=(g == 0),
                            stop=(g == 1),
                            perf_mode=DR,
                        )
                        for d2 in range(2):
                            nc.tensor.matmul(
                                tg[d2],
                                pnq_sb[g][:, :, d2 * 128 : (d2 + 1) * 128],
                                w1q[g],
                                start=(g == 0),
                                stop=(g == 1),
                                perf_mode=DR,
                            )
                s["tg"] = tg
                # isn = 0.5/s1 (from 2*s1 row), broadcast to all partitions
                isn = isnp.tile([1, MACRO], bf16, tag="isn")
                recip_fast(nc, isn, rows[0:1, :])
                bcq = sq_ps.tile([128, MACRO], f32, tag="sq")
                nc.tensor.matmul(bcq, onesrow_sb, isn, start=True, stop=True)
                bcs = bcsp.tile([128, MACRO], bf16, tag="bcs")
                nc.scalar.copy(out=bcs, in_=bcq)
                # x^T = 0.5 v^T + (0.5/s1) * target^T
                xt = []
                for d2 in range(2):
                    th = thp.tile([128, MACRO], f32, tag="th")
                    nc.vector.tensor_mul(th, tg[d2], bcs)
                    xtt = xtp.tile([128, MACRO], bf16, tag="xt")
                    nc.vector.tensor_add(xtt, th, vsl[d2])
                    xt.append(xtt)
                s["xt"] = xt

            def mid(im):
                s = st[im]
                ev = im % 2
                rows, xt = s["rows"], s["xt"]
                # x2 row
                for d2 in range(2):
                    sq = sqvp.tile([128, MACRO], bf16, tag="sqv")
                    nc.scalar.square(sq, xt[d2])
                    nc.tensor.matmul(
                        rows[32:33, :],
                        ones_col,
                        sq,
                        start=(d2 == 0),
                        stop=(d2 == 1),
                    )
                nc.scalar.copy(out=aug2l_sb[ev][0:1, :], in_=rows[32:33, :])

            def back(im):
                s = st[im]
                tok0 = im * MACRO
                ev = im % 2
                xt = s["xt"]
                # dist2 (orientation A), cross matmuls ahead of aug ones
                ps2 = [
                    ps2_ps.tile([128, K], f32, tag="ps2", name=f"ps2_{si}")
                    for si in range(4)
                ]
                ob4 = obp.tile([128, 4, K], out_dtype, tag="ob")
                w2s = []
                s2c4 = smallp.tile([128, 4], f32, tag="s2c4")
                inv4 = smallp.tile([128, 4], f32, tag="inv4")

                def cross(si):
                    for d2 in range(2):
                        nc.tensor.matmul(
                            ps2[si],
                            xt[d2][:, si * 128 : (si + 1) * 128],
                            ptm2_sb[d2],
                            start=(d2 == 0),
                            stop=False,
                        )

                def finish(si):
                    nc.tensor.matmul(
                        ps2[si],
                        aug2l_sb[ev][:, si * 128 : (si + 1) * 128],
                        aug2r_sb,
                        start=False,
                        stop=True,
                    )
                    w2 = w2p.tile([128, K], f32, tag="w2")
                    recip_acc(nc, w2, ps2[si], accum_out=s2c4[:, si : si + 1])
                    w2s.append(w2)

                cross(0)
                cross(1)
                finish(0)
                cross(2)
                finish(1)
                cross(3)
                finish(2)
                finish(3)
                nc.vector.reciprocal_approx_fast(out=inv4, in_=s2c4)
                for si in range(4):
                    nc.scalar.activation(
                        out=ob4[:, si, :],
                        in_=w2s[si],
                        func=FT.Copy,
                        scale=inv4[:, si : si + 1],
                    )
                nc.sync.dma_start(
                    out=out_d[tok0 : tok0 + MACRO, :].rearrange(
                        "(s p) k -> p s k", p=128
                    ),
                    in_=ob4,
                )

            front(0)
            front(1)
            mid(0)
            for im in range(nmacro):
                if im + 2 < nmacro:
                    front(im + 2)
                if im + 1 < nmacro:
                    mid(im + 1)
                back(im)
    if do_compile:
        nc.compile()
    return nc


def static_inputs(protos):
    import ml_dtypes

    b = ml_dtypes.bfloat16
    f8 = ml_dtypes.float8_e4m3
    protos = np.ascontiguousarray(protos, dtype=np.float32)
    pt = protos.T  # [D, K]
    c2 = (protos * protos).sum(axis=1).astype(np.float32)  # [K]
    # dist1 runs at 1/64 scale so w1' = 64/sq1 sits in fp8's sweet spot
    aug1l = np.stack([np.full(K, 4.0 / 64.0, np.float32), c2 / 64.0])
    aug2r = np.stack([np.ones(K, np.float32), c2])
    rowinit = np.stack([np.zeros(MACRO, np.float32), np.ones(MACRO, np.float32)])
    consts = np.stack(
        [np.ones(128, np.float32), np.full(128, 2.0, np.float32)], axis=1
    )
    onesrow = np.ones((1, 128), np.float32)
    # DoubleRow pair layouts: [p, i, *] = contraction row i*128 + p
    ptm4q = np.ascontiguousarray((-0.25 * pt).reshape(2, 128, K).transpose(1, 0, 2))
    pnq = np.ascontiguousarray((4.0 * protos).reshape(2, 2, 128, D).transpose(0, 2, 1, 3))
    consts8 = np.full((128, 2, 16), 8.0, np.float32)
    return {
        "ptm4q": ptm4q.astype(f8),
        "pnq": pnq.astype(f8),
        "consts8": consts8.astype(f8),
        "ptm2": np.ascontiguousarray(-2.0 * pt).astype(b),
        "aug1l": np.ascontiguousarray(aug1l).astype(b),
        "aug2r": np.ascontiguousarray(aug2r).astype(b),
        "rowinit": np.ascontiguousarray(rowinit).astype(b),
        "consts": np.ascontiguousarray(consts).astype(b),
        "onesrow": onesrow.astype(b),
    }


_NC_CACHE = {}


def _get_nc(T):
    if T not in _NC_CACHE:
        _NC_CACHE[T] = build_bass(T)
    return _NC_CACHE[T]


def _run(encodedData, protos, trace=False):
    import ml_dtypes
    from concourse.bass_utils import run_bass_kernel_spmd

    b = ml_dtypes.bfloat16
    enc = np.ascontiguousarray(np.asarray(encodedData, dtype=np.float32))
    assert enc.shape == (B, N, D)
    T = (B // NCORES) * N
    nc = _get_nc(T)
    statics = static_inputs(np.asarray(protos, dtype=np.float32))
    bloc = B // NCORES
    in_maps = []
    f8 = ml_dtypes.float8_e4m3
    for c in range(NCORES):
        ec = enc[c * bloc : (c + 1) * bloc].reshape(T, D)
        ecT = ec.T  # [D, T]
        vth = (0.5 * ecT).astype(b).reshape(2, 128, T)
        vthq = np.ascontiguousarray(
            (0.125 * ecT).reshape(2, 128, T).transpose(1, 0, 2)
        ).astype(f8)
        v2q = 0.25 * (ec * ec).sum(axis=1)
        aug1r = np.stack([v2q, np.ones(T, np.float32)]).astype(b)
        in_maps.append(
            {
                "vth": np.ascontiguousarray(vth),
                "vthq": vthq,
                "aug1r": np.ascontiguousarray(aug1r),
                **statics,
            }
        )
    res = run_bass_kernel_spmd(nc, in_maps, core_ids=list(range(NCORES)), trace=trace)
    out = np.empty((B, N, K), np.float32)
    for c in range(NCORES):
        out[c * bloc : (c + 1) * bloc] = (
            res.results[c]["out"].astype(np.float32).reshape(bloc, N, K)
        )
    return out, res


def kernel(**inputs):
    out, _ = _run(inputs["encodedData"], inputs["protos"])
    return out


def kernel_profiled(**inputs):
    out, res = _run(inputs["encodedData"], inputs["protos"], trace=True)
    return out, res


# revision 23
# speedup vs baseline: 1.0367x; 1.0367x over previous
"""Trainium2 Bass kernel for nn_CPCModel_50878182588587 (vq_codebook).

Computes, for inputs encodedData [B,N,D] and protos [K,D]:
  pass1: FCM memberships of v vs protos (p=2), x = 0.5*v + 0.5*(belong@protos)
  pass2: FCM memberships of x vs protos (p=2)  -> output [B,N,K]

Sharding: data-parallel over B across 8 NeuronCores; protos replicated.

Design (per core, T=8192 tokens, macro-tiles of 512 tokens):
  - Host supplies vth = 0.5*v^T in bf16 (layout prep) and the aug row
    [||v||^2/4 ; 1], so the kernel has no transposes / input squares.
  - sq = ||v||^2 + ||c||^2 - 2 v.c formed entirely in PSUM via
    augmented-contraction matmul rows; 1/sq via DVE reciprocal_approx_fast
    (sq bounded away from 0 here: sq1 in [170,351], sq2 in [42,91]).
  - Orientation B (K/D on partitions, tokens free) for dist1/target;
    orientation A (tokens on partitions) for dist2 so s2 reduction and
    the final normalize are per-partition ops.
  - pass-2 reciprocal is a registered custom DVE op (1-Newton recip with
    free-dim accumulate) producing w2 AND s2 in one instruction; final
    normalize on the Activation engine (Copy with per-partition scale).
    The Pool/GpSimd engine is pathologically slow and unused.
  - Issue order is software-pipelined three stages deep:
    F(im) = dist1+srow+tgt+bcq+bcs+xt, M(im) = x2row/aug2l,
    B(im) = dist2/recip/normalize/out, issued F0,F1,M0,F2,M1,B0,... so
    TensorE never waits on the cross-engine reduction chains.
  - DMA: macro-0's dependencies are loaded first (small leading chunks);
    output is written in the on-chip [im, p, s, k] layout (4KB
    descriptors) and unpermuted on host.  Output in bf16 (harness gate is
    2e-2; bf16 adds ~2e-3 max), host casts back to f32.
"""

import sys

import numpy as np

sys.path.insert(0, "/opt/trn_rl_repo")

import concourse.bass as bass  # noqa: E402
from concourse import bacc  # noqa: E402
import concourse.mybir as mybir  # noqa: E402
import concourse.tile as tile  # noqa: E402

B, N, D, K = 64, 1024, 256, 512
NCORES = 8
MACRO = 512  # tokens per macro-tile
f32 = mybir.dt.float32
bf16 = mybir.dt.bfloat16
FT = mybir.ActivationFunctionType


def recip_fast(nc, out, in_):
    """reciprocal_approx_fast with any output dtype (wrapper asserts fp32)."""
    from concourse.dve_ops import RECIP_APPROX_FAST_CONSTS, RECIPROCAL_APPROX_FAST

    c = RECIP_APPROX_FAST_CONSTS
    return nc.vector._custom_dve(
        RECIPROCAL_APPROX_FAST, out=out, in0=in_, s0=c["s0"], s1=c["s1"], imm2=c["imm2"]
    )


# one-Newton approximate reciprocal with free-dim accumulate: out = ~1/in0,
# accum_out = sum(out).  Seed constants tuned for the [-4.5,-4] interval of
# x*bitcast(~x); max rel err ~2e-3 -- fine for the normalize-only use.
RECIP1NR_C0 = -4.0 / 17.0 * 1.004
RECIP1NR_C1 = 2.0 * 1.001


def _register_recip_acc():
    from operator import add as _add

    from concourse import dve_ops as D
    from concourse.dve_spec import C0, C1, AluOp, Bin, Spec, Src0, Zero, lower
    from concourse.dve_uop import DveOpSpec

    for op in D.OPS:
        if op.name == "RECIP1NR_ACC_ANT":
            return op

    def _ref(in0, in1, c0, c1, c2):
        import numpy as np

        nx = (~np.asarray(in0, np.float32).view(np.int32)).view(np.float32)
        y0 = nx * c0
        body = y0 * (c1 - in0 * y0)
        return body, body.reshape(body.shape[0], -1).sum(axis=-1)

    _not = Bin(AluOp.BITWISE_NOT, Src0, Src0)
    _y0 = _not * C0
    spec = Spec(
        body=_y0 * (C1 - Src0 * _y0),
        accum=_add,
        accum_init=Zero,
        reference=_ref,
    )
    row = max(D._SUB_OPCODE_FOR_NAME.values()) + 1
    name = "RECIP1NR_ACC_ANT"
    uops = lower(spec, ver="v3")
    sha = DveOpSpec(name=name, opcode=row, uops=uops, rd1_en=False).sha("v3")
    op = D.DveOp(name, spec, subdim=False, uops_sha={"v3": sha})
    D.OPS.append(op)
    D.CUSTOM_DVE_SPECS[name] = spec
    D._SUB_OPCODE_FOR_NAME[name] = row
    return op


def recip_acc(nc, out, in_, accum_out):
    op = _register_recip_acc()
    return nc.vector._custom_dve(
        op,
        out=out,
        in0=in_,
        s0=RECIP1NR_C0,
        s1=RECIP1NR_C1,
        imm2=0.0,
        accum_out=accum_out,
    )


def build_bass(T, do_compile=True, out_dtype=bf16):
    assert T % MACRO == 0
    nmacro = T // MACRO
    nc = bacc.Bacc(trn_type="TRN2")

    vth_d = nc.dram_tensor("vth", [2, 128, T], bf16, kind="ExternalInput")  # 0.5 v^T
    aug1r_d = nc.dram_tensor("aug1r", [2, T], bf16, kind="ExternalInput")  # v2q;1
    ptm4_d = nc.dram_tensor("ptm4", [D, K], bf16, kind="ExternalInput")  # -4*protos.T
    ptm2_d = nc.dram_tensor("ptm2", [D, K], bf16, kind="ExternalInput")  # -2*protos.T
    pn_d = nc.dram_tensor("pn", [K, D], bf16, kind="ExternalInput")  # protos
    # aug1l rows: [0]=4.0 (scales v2q back to v2), [1]=c2
    aug1l_d = nc.dram_tensor("aug1l", [2, K], bf16, kind="ExternalInput")
    # aug2r rows: [0]=1.0 (x2 row), [1]=c2
    aug2r_d = nc.dram_tensor("aug2r", [2, K], bf16, kind="ExternalInput")
    rowinit_d = nc.dram_tensor("rowinit", [2, MACRO], bf16, kind="ExternalInput")
    consts_d = nc.dram_tensor("consts", [128, 2], bf16, kind="ExternalInput")  # 1s, 2s
    onesrow_d = nc.dram_tensor("onesrow", [1, 128], bf16, kind="ExternalInput")
    # on-chip layout: token t = im*512 + s*128 + p  ->  out[im, p, s, :]
    out_d = nc.dram_tensor("out", [nmacro, 128, 4, K], out_dtype, kind="ExternalOutput")

    with tile.TileContext(nc) as tc:
        with (
            tc.tile_pool(name="singles", bufs=1) as singles,
            tc.tile_pool(name="wt", bufs=6) as wtp,
            tc.tile_pool(name="bcs", bufs=3) as bcsp,
            tc.tile_pool(name="isn", bufs=2) as isnp,
            tc.tile_pool(name="th", bufs=2) as thp,
            tc.tile_pool(name="xt", bufs=6) as xtp,
            tc.tile_pool(name="sqv", bufs=2) as sqvp,
            tc.tile_pool(name="w2", bufs=4) as w2p,
            tc.tile_pool(name="ob", bufs=3) as obp,
            tc.tile_pool(name="small", bufs=16) as smallp,
            tc.tile_pool(name="sqp", bufs=2, space="PSUM") as sq_ps,
            tc.tile_pool(name="tgp", bufs=2, space="PSUM") as tg_ps,
            tc.tile_pool(name="ps2p", bufs=3, space="PSUM") as ps2_ps,
            tc.tile_pool(name="rwp", bufs=1, space="PSUM") as rows_ps,
        ):
            # ---- statics + input, ordered so macro 0's deps land first ----
            consts_sb = singles.tile([128, 2], bf16, tag="consts")
            nc.sync.dma_start(out=consts_sb, in_=consts_d[:, :])
            onesrow_sb = singles.tile([1, 128], bf16, tag="onesrow")
            nc.sync.dma_start(out=onesrow_sb, in_=onesrow_d[:, :])
            aug1l_sb = singles.tile([2, K], bf16, tag="aug1l")
            nc.sync.dma_start(out=aug1l_sb, in_=aug1l_d[:, :])
            aug1r_sb = singles.tile([2, T], bf16, tag="aug1r")
            nc.sync.dma_start(out=aug1r_sb, in_=aug1r_d[:, :])
            vth_sb = [
                singles.tile([128, T], bf16, tag=f"vth_{d2}", name=f"vth_{d2}")
                for d2 in range(2)
            ]
            # leading chunks small so the first macros' reads unblock early
            bounds = [0, 512, 1024, 2048, 4096, T]
            for ci in range(len(bounds) - 1):
                lo, hi = bounds[ci], bounds[ci + 1]
                if ci == 1:
                    # macro 0 can start; now its front-phase statics
                    ptm4_sb = []
                    for d2 in range(2):
                        t4 = singles.tile(
                            [128, K], bf16, tag=f"ptm4_{d2}", name=f"ptm4_{d2}"
                        )
                        nc.sync.dma_start(
                            out=t4, in_=ptm4_d[d2 * 128 : (d2 + 1) * 128, :]
                        )
                        ptm4_sb.append(t4)
                    pn_sb = []
                    for kc in range(4):
                        t = singles.tile([128, D], bf16, tag=f"pn_{kc}", name=f"pn_{kc}")
                        nc.sync.dma_start(out=t, in_=pn_d[kc * 128 : (kc + 1) * 128, :])
                        pn_sb.append(t)
                if ci == 2:
                    aug2r_sb = singles.tile([2, K], bf16, tag="aug2r")
                    nc.sync.dma_start(out=aug2r_sb, in_=aug2r_d[:, :])
                    aug2l_sb = []
                    for e in range(2):
                        t = singles.tile(
                            [2, MACRO], bf16, tag=f"aug2l_{e}", name=f"aug2l_{e}"
                        )
                        nc.sync.dma_start(out=t, in_=rowinit_d[:, :])
                        aug2l_sb.append(t)
                    ptm2_sb = []
                    for d2 in range(2):
                        t2 = singles.tile(
                            [128, K], bf16, tag=f"ptm2_{d2}", name=f"ptm2_{d2}"
                        )
                        nc.sync.dma_start(
                            out=t2, in_=ptm2_d[d2 * 128 : (d2 + 1) * 128, :]
                        )
                        ptm2_sb.append(t2)
                for d2 in range(2):
                    nc.sync.dma_start(
                        out=vth_sb[d2][:, lo:hi], in_=vth_d[d2, :, lo:hi]
                    )
            ones_col = consts_sb[:, 0:1]
            twos_col = consts_sb[:, 1:2]

            st = [dict() for _ in range(nmacro)]  # per-macro state

            def front(im):
                s = st[im]
                tok0 = im * MACRO
                vsl = [vth_sb[d2][:, tok0 : tok0 + MACRO] for d2 in range(2)]
                # rows psum: [0:1]=2*s1 row, [32:33]=x2 row (written in mid)
                rows = rows_ps.tile([33, MACRO], f32, tag="rows")
                s["rows"] = rows
                # dist1 + w1 per k-chunk (orientation B); srow accumulates 2*s1
                wt = []
                for kc in range(4):
                    sqp = sq_ps.tile([128, MACRO], f32, tag="sq")
                    for d2 in range(2):
                        nc.tensor.matmul(
                            sqp,
                            ptm4_sb[d2][:, kc * 128 : (kc + 1) * 128],
                            vsl[d2],
                            start=(d2 == 0),
                            stop=False,
                        )
                    nc.tensor.matmul(
                        sqp,
                        aug1l_sb[:, kc * 128 : (kc + 1) * 128],
                        aug1r_sb[:, tok0 : tok0 + MACRO],
                        start=False,
                        stop=True,
                    )
                    w = wtp.tile([128, MACRO], bf16, tag="wt")
                    recip_fast(nc, w, sqp)
                    nc.tensor.matmul(
                        rows[0:1, :],
                        twos_col,
                        w,
                        start=(kc == 0),
                        stop=(kc == 3),
                    )
                    wt.append(w)
                # target^T (orientation B)
                tg = []
                for d2 in range(2):
                    ps = tg_ps.tile([128, MACRO], f32, tag="tg")
                    for kc in range(4):
                        nc.tensor.matmul(
                            ps,
                            pn_sb[kc][:, d2 * 128 : (d2 + 1) * 128],
                            wt[kc],
                            start=(kc == 0),
                            stop=(kc == 3),
                        )
                    tg.append(ps)
                # isn = 0.5/s1 (from 2*s1 row), broadcast to all partitions
                isn = isnp.tile([1, MACRO], bf16, tag="isn")
                recip_fast(nc, isn, rows[0:1, :])
                bcq = sq_ps.tile([128, MACRO], f32, tag="sq")
                nc.tensor.matmul(bcq, onesrow_sb, isn, start=True, stop=True)
                bcs = bcsp.tile([128, MACRO], bf16, tag="bcs")
                nc.scalar.copy(out=bcs, in_=bcq)
                # x^T = 0.5 v^T + (0.5/s1) * target^T
                xt = []
                for d2 in range(2):
                    th = thp.tile([128, MACRO], f32, tag="th")
                    nc.vector.tensor_mul(th, tg[d2], bcs)
                    xtt = xtp.tile([128, MACRO], bf16, tag="xt")
                    nc.vector.tensor_add(xtt, th, vsl[d2])
                    xt.append(xtt)
                s["xt"] = xt

            def mid(im):
                s = st[im]
                ev = im % 2
                rows, xt = s["rows"], s["xt"]
                # x2 row
                for d2 in range(2):
                    sq = sqvp.tile([128, MACRO], bf16, tag="sqv")
                    nc.scalar.square(sq, xt[d2])
                    nc.tensor.matmul(
                        rows[32:33, :],
                        ones_col,
                        sq,
                        start=(d2 == 0),
                        stop=(d2 == 1),
                    )
                nc.scalar.copy(out=aug2l_sb[ev][0:1, :], in_=rows[32:33, :])

            def back(im):
                s = st[im]
                ev = im % 2
                xt = s["xt"]
                # dist2 (orientation A), cross matmuls ahead of aug ones
                ps2 = [
                    ps2_ps.tile([128, K], f32, tag="ps2", name=f"ps2_{si}")
                    for si in range(4)
                ]
                ob4 = obp.tile([128, 4, K], out_dtype, tag="ob")
                w2s = []
                s2c4 = smallp.tile([128, 4], f32, tag="s2c4")
                inv4 = smallp.tile([128, 4], f32, tag="inv4")

                def cross(si):
                    for d2 in range(2):
                        nc.tensor.matmul(
                            ps2[si],
                            xt[d2][:, si * 128 : (si + 1) * 128],
                            ptm2_sb[d2],
                            start=(d2 == 0),
                            stop=False,
                        )

                def finish(si):
                    nc.tensor.matmul(
                        ps2[si],
                        aug2l_sb[ev][:, si * 128 : (si + 1) * 128],
                        aug2r_sb,
                        start=False,
                        stop=True,
                    )
                    w2 = w2p.tile([128, K], f32, tag="w2")
                    recip_acc(nc, w2, ps2[si], accum_out=s2c4[:, si : si + 1])
                    w2s.append(w2)

                cross(0)
                cross(1)
                finish(0)
                cross(2)
                finish(1)
                cross(3)
                finish(2)
                finish(3)
                nc.vector.reciprocal_approx_fast(out=inv4, in_=s2c4)
                for si in range(4):
                    nc.scalar.activation(
                        out=ob4[:, si, :],
                        in_=w2s[si],
                        func=FT.Copy,
                        scale=inv4[:, si : si + 1],
                    )
                nc.sync.dma_start(out=out_d[im], in_=ob4)

            front(0)
            front(1)
            mid(0)
            for im in range(nmacro):
                if im + 2 < nmacro:
                    front(im + 2)
                if im + 1 < nmacro:
                    mid(im + 1)
                back(im)
    if do_compile:
        nc.compile()
    return nc


def static_inputs(protos):
    import ml_dtypes

    b = ml_dtypes.bfloat16
    protos = np.ascontiguousarray(protos, dtype=np.float32)
    pt = protos.T  # [D, K]
    c2 = (protos * protos).sum(axis=1).astype(np.float32)  # [K]
    aug1l = np.stack([np.full(K, 4.0, np.float32), c2])
    aug2r = np.stack([np.ones(K, np.float32), c2])
    rowinit = np.stack([np.zeros(MACRO, np.float32), np.ones(MACRO, np.float32)])
    consts = np.stack(
        [np.ones(128, np.float32), np.full(128, 2.0, np.float32)], axis=1
    )
    onesrow = np.ones((1, 128), np.float32)
    return {
        "ptm4": np.ascontiguousarray(-4.0 * pt).astype(b),
        "ptm2": np.ascontiguousarray(-2.0 * pt).astype(b),
        "pn": protos.astype(b),
        "aug1l": np.ascontiguousarray(aug1l).astype(b),
        "aug2r": np.ascontiguousarray(aug2r).astype(b),
        "rowinit": np.ascontiguousarray(rowinit).astype(b),
        "consts": np.ascontiguousarray(consts).astype(b),
        "onesrow": onesrow.astype(b),
    }


_NC_CACHE = {}


def _get_nc(T):
    if T not in _NC_CACHE:
        _NC_CACHE[T] = build_bass(T)
    return _NC_CACHE[T]


def _run(encodedData, protos, trace=False):
    import ml_dtypes
    from concourse.bass_utils import run_bass_kernel_spmd

    b = ml_dtypes.bfloat16
    enc = np.ascontiguousarray(np.asarray(encodedData, dtype=np.float32))
    assert enc.shape == (B, N, D)
    T = (B // NCORES) * N
    nc = _get_nc(T)
    statics = static_inputs(np.asarray(protos, dtype=np.float32))
    bloc = B // NCORES
    in_maps = []
    for c in range(NCORES):
        ec = enc[c * bloc : (c + 1) * bloc].reshape(T, D)
        vth = (0.5 * ec.T).astype(b).reshape(2, 128, T)
        v2q = 0.25 * (ec * ec).sum(axis=1)
        aug1r = np.stack([v2q, np.ones(T, np.float32)]).astype(b)
        in_maps.append(
            {
                "vth": np.ascontiguousarray(vth),
                "aug1r": np.ascontiguousarray(aug1r),
                **statics,
            }
        )
    res = run_bass_kernel_spmd(nc, in_maps, core_ids=list(range(NCORES)), trace=trace)
    out = np.empty((B, N, K), np.float32)
    for c in range(NCORES):
        # device layout [im, p, s, k]; token t = im*512 + s*128 + p
        oc = res.results[c]["out"].astype(np.float32)
        out[c * bloc : (c + 1) * bloc] = oc.transpose(0, 2, 1, 3).reshape(bloc, N, K)
    return out, res


def kernel(**inputs):
    out, _ = _run(inputs["encodedData"], inputs["protos"])
    return out


def kernel_profiled(**inputs):
    out, res = _run(inputs["encodedData"], inputs["protos"], trace=True)
    return out, res
